# revision 28
# baseline (speedup 1.0000x reference)
"""Trainium2 Bass kernel for nn_CAT_LSTM (TLSTM with stage-embedding MLP).

Sharding: data-parallel over batch across 8 NeuronCores (b=16 per core).

All on-chip compute is *feature-major* (features on SBUF partitions):
every matmul uses natural-layout stationary weights (lhsT = W) with
feature-major moving activations; the host pre-transposes x/stages per
core with row-columns ordered (s, b), and pre-casts matmul operands to
fp16 (identical numerics to an on-device cast, no staging needed).

Per core, one fused schedule (Tile interleaves by dependencies):
  GEMM stream (row-chunked): stages^T -MLP-> st3^T;
      u^T chunk = U_all^T @ [x^T; st3^T] + (U_all_b + W_all_b),
      kept in SBUF (pool bufs=3) and consumed directly by the scan.
  Scan (S=256 steps, batch=16): gate-major packed layout
      [128 part, 8 gate-tiles x 16 batch]; stationary-weight fp16
      matmuls; W_d bias folded in as a rank-1 (K=1) matmul; elementwise
      on [128,32] tiles; h streamed to featT; classifier per chunk.

Matmul dtype: fp16 (10-bit mantissa; end-to-end rel err ~1e-3).
"""
import numpy as np

import concourse.bass as bass
import concourse.bacc as bacc
import concourse.mybir as mybir
import concourse.tile as tile
from concourse import bass_utils

B, S, H = 128, 256, 256
NCORES = 8
BC = B // NCORES          # 16 batch rows per core
R = S * BC                # 4096 (s,b) rows per core
G4 = 4 * H                # 1024
DX = 1024
DST = 67
DIN = 2048

f32 = mybir.dt.float32
bf16 = mybir.dt.bfloat16
fp16 = mybir.dt.float16
AF = mybir.ActivationFunctionType

DEFAULT_CFG = dict(
    nc_chunk=512,          # GEMM column chunk
    mm_dtype="fp16",       # matmul dtype: "fp16" | "bf16"
    rep=1,                 # repeat body (timing)
    phases="all",          # "all" | "gemm" (u-chunks only) — scan needs gemm
)


def _np_mmdt(cfg):
    if cfg["mm_dtype"] == "fp16":
        return np.float16
    import ml_dtypes
    return ml_dtypes.bfloat16


def _build(cfg=None):
    cfg = dict(DEFAULT_CFG, **(cfg or {}))
    mdt = {"fp16": fp16, "bf16": bf16}[cfg["mm_dtype"]]

    nc = bacc.Bacc("TRN2", target_bir_lowering=False, debug=False)

    dram = {}
    for name, shape, dt in [
        ("xT", [DX, R], mdt), ("stT", [DST, R], mdt), ("ts1", [S, BC], f32),
        ("w1", [DST, 256], mdt), ("b1", [256], f32),
        ("w2", [256, 512], mdt), ("b2", [512], f32),
        ("w3", [512, 1024], mdt), ("b3", [1024], f32),
        ("uw", [DIN, G4], mdt), ("ub", [G4], f32),
        ("waw", [H, G4], mdt), ("wdw", [H, H], mdt), ("wdb", [H], mdt),
        ("clw", [H, 1], mdt), ("clb", [1], f32),
    ]:
        dram[name] = nc.dram_tensor(name, shape, dt, kind="ExternalInput").ap()
    featT = nc.dram_tensor("featT", [128, 2 * R], f32, kind="ExternalOutput").ap()
    outT = nc.dram_tensor("outT", [1, R], f32, kind="ExternalOutput").ap()

    import contextlib
    with tile.TileContext(nc) as tc:
        with contextlib.ExitStack() as ctx:
            _body(ctx, tc, nc, cfg, mdt, dram, featT, outT)
    nc.compile()
    return nc


GPERM = [0, 1, 2, 3, 6, 7, 4, 5]   # slots: f f i i g g o o


def _body(ctx, tc, nc, cfg, mdt, dram, featT, outT):
    NC_ = cfg["nc_chunk"]
    n_chunks = R // NC_
    spc = NC_ // BC                      # steps per chunk
    d = dram

    wpool = ctx.enter_context(tc.tile_pool(name="wpool", bufs=1))
    xpool = ctx.enter_context(tc.tile_pool(name="xpool", bufs=3))
    mpool = ctx.enter_context(tc.tile_pool(name="mpool", bufs=1))
    upool = ctx.enter_context(tc.tile_pool(name="upool", bufs=4))
    gpsum = ctx.enter_context(tc.tile_pool(name="gpsum", bufs=4, space="PSUM"))
    spool = ctx.enter_context(tc.tile_pool(name="spool", bufs=2))
    spsum = ctx.enter_context(tc.tile_pool(name="spsum", bufs=1, space="PSUM"))
    gsum2 = ctx.enter_context(tc.tile_pool(name="gsum2", bufs=2, space="PSUM"))
    fpool = ctx.enter_context(tc.tile_pool(name="fpool", bufs=2))
    frpool = ctx.enter_context(tc.tile_pool(name="frpool", bufs=2))

    # ---- weights: direct DMA (host pre-cast) ----
    def load_w(ap_dram, K, M, tag):
        kt = max(1, K // 128)
        P = min(K, 128)
        t = wpool.tile([P, kt, M], mdt, tag=tag)
        src = (ap_dram.unsqueeze(0) if kt == 1
               else ap_dram.rearrange("(k p) m -> k p m", p=128))
        for k in range(kt):
            nc.sync.dma_start(out=t[:, k], in_=src[k])
        return t

    w1s = load_w(d["w1"], DST, 256, "w1")
    str_ = load_w(d["stT"], DST, R, "st")

    def load_bias(ap_dram, M, tag):
        mt = (M + 127) // 128
        t = wpool.tile([min(M, 128), mt], f32, tag=tag)
        if mt == 1:
            nc.sync.dma_start(out=t, in_=ap_dram.unsqueeze(1))
        else:
            nc.sync.dma_start(out=t, in_=ap_dram.rearrange("(m p) -> p m", p=128))
        return t

    b1s = load_bias(d["b1"], 256, "b1")
    b2s = load_bias(d["b2"], 512, "b2")
    b3s = load_bias(d["b3"], 1024, "b3")
    ubs = load_bias(d["ub"], G4, "ub")
    clbs = load_bias(d["clb"], 1, "clb")
    w2s = load_w(d["w2"], 256, 512, "w2")
    w3s = load_w(d["w3"], 512, 1024, "w3")
    was = load_w(d["waw"], H, G4, "wa")
    wds = load_w(d["wdw"], H, H, "wd")
    clws = load_w(d["clw"], H, 1, "clw")

    # ones row + W_d bias row for the rank-1 bias matmul
    ones_r = wpool.tile([1, 128], mdt, tag="ones")
    ones_f = wpool.tile([1, 128], f32, tag="ones_f")
    nc.vector.memset(ones_f, 1.0)
    nc.vector.tensor_copy(out=ones_r, in_=ones_f)
    wdb_row = wpool.tile([1, H], mdt, tag="wdb_row")
    nc.sync.dma_start(out=wdb_row, in_=d["wdb"].unsqueeze(0))

    # warm the sigmoid/tanh ACT table set before the first Relu
    warm = wpool.tile([1, 16], f32, tag="warm")
    nc.scalar.activation(out=warm, in_=ones_f[:, 0:16], func=AF.Sigmoid)
    nc.scalar.activation(out=warm, in_=warm, func=AF.Tanh)

    # (tt - 1) broadcast over partitions, all steps resident: [128, S, BC]
    tsb_all = wpool.tile([128, S, BC], f32, tag="tsb")
    nc.sync.dma_start(
        out=tsb_all[:, 0:32],
        in_=bass.AP(tensor=d["ts1"].tensor, offset=d["ts1"].offset,
                    ap=[[0, 128], [1, 32 * BC]]))
    uws = load_w(d["uw"], DIN, G4, "uw")
    nc.sync.dma_start(
        out=tsb_all[:, 32:],
        in_=bass.AP(tensor=d["ts1"].tensor, offset=d["ts1"].offset + 32 * BC,
                    ap=[[0, 128], [1, (S - 32) * BC]]))

    xT_v = d["xT"].rearrange("(k p) r -> p k r", p=128)

    def make_chunk_ops(c, cs, W, ucs):
        """Build a GEMM chunk (cols [cs, cs+W)) as a list of closures."""
        st = {}
        ops = []

        def op_xdma():
            st["xc"] = xpool.tile([128, 8, W], mdt, tag="xc", name=f"xc{c}")
            nc.sync.dma_start(out=st["xc"], in_=xT_v[:, :, cs:cs + W])
            st["st1"] = mpool.tile([128, 2, W], mdt, tag="st1", name=f"st1_{c}")
        ops.append(op_xdma)

        def mk_l1(m):
            def f():
                ps = gpsum.tile([128, W], f32, tag="ps")
                nc.tensor.matmul(ps, w1s[:, 0, m * 128:(m + 1) * 128],
                                 str_[:, 0, cs:cs + W], start=True, stop=True)
                nc.scalar.activation(out=st["st1"][:, m], in_=ps, func=AF.Relu,
                                     bias=b1s[:, m:m + 1], scale=1.0)
            return f
        ops += [mk_l1(m) for m in range(2)]

        def op_alloc2():
            st["st2"] = mpool.tile([128, 4, W], mdt, tag="st2", name=f"st2_{c}")
        ops.append(op_alloc2)

        def mk_l2(m):
            def f():
                ps = gpsum.tile([128, W], f32, tag="ps")
                for k in range(2):
                    nc.tensor.matmul(
                        ps, w2s[:, k, m * 128:(m + 1) * 128],
                        st["st1"][:, k], start=(k == 0), stop=(k == 1))
                nc.scalar.activation(out=st["st2"][:, m], in_=ps, func=AF.Relu,
                                     bias=b2s[:, m:m + 1], scale=1.0)
            return f
        ops += [mk_l2(m) for m in range(4)]

        def op_alloc3():
            st["st3"] = mpool.tile([128, 8, W], mdt, tag="st3", name=f"st3_{c}")
        ops.append(op_alloc3)

        def mk_l3(m):
            def f():
                ps = gpsum.tile([128, W], f32, tag="ps")
                for k in range(4):
                    nc.tensor.matmul(
                        ps, w3s[:, k, m * 128:(m + 1) * 128],
                        st["st2"][:, k], start=(k == 0), stop=(k == 3))
                nc.scalar.activation(out=st["st3"][:, m], in_=ps, func=AF.Relu,
                                     bias=b3s[:, m:m + 1], scale=1.0)
            return f
        ops += [mk_l3(m) for m in range(8)]

        def op_allocu():
            uc = upool.tile([128, 8, W], f32, tag="uc", name=f"uc{c}")
            st["uc"] = uc
            ucs[c] = uc
        ops.append(op_allocu)

        # gate-tile permutation: psG/u slot m holds W_all column block GPERM[m]
        def mk_u(m, khalf):
            def f():
                ps = st.setdefault(("psu", m), None)
                if khalf == 0:
                    ps = gpsum.tile([128, W], f32, tag="ps")
                    st[("psu", m)] = ps
                else:
                    ps = st[("psu", m)]
                for k in range(khalf * 8, khalf * 8 + 8):
                    rhs = st["xc"][:, k] if k < 8 else st["st3"][:, k - 8]
                    gm = GPERM[m]
                    nc.tensor.matmul(
                        ps, uws[:, k, gm * 128:(gm + 1) * 128],
                        rhs, start=(k == 0), stop=(k == 15))
                if khalf == 1:
                    nc.scalar.activation(out=st["uc"][:, m], in_=ps,
                                         func=AF.Identity,
                                         bias=ubs[:, m:m + 1], scale=1.0)
            return f
        ops += [mk_u(m, kh) for m in range(8) for kh in (0, 1)]
        return ops

    for _rep in range(cfg["rep"]):
        ucs = {}
        # chunk table: small warm-up chunks so the scan starts early
        bounds = []
        pos = 0
        for w in [128, 192, 288, 432]:
            bounds.append((pos, w)); pos += w
        while R - pos > NC_:
            bounds.append((pos, NC_)); pos += NC_
        if R - pos:
            bounds.append((pos, R - pos)); pos = R
        nch = len(bounds)
        s2c = []                       # step -> chunk index
        c1st = []                      # chunk -> first step
        for ci, (rs, rn) in enumerate(bounds):
            c1st.append(len(s2c))
            s2c += [ci] * (rn // BC)
        # chunk 0 before the scan; chunk k spread over chunk k-1's steps
        for op in make_chunk_ops(0, bounds[0][0], bounds[0][1], ucs):
            op()
        emit_at = [[] for _ in range(S)]
        for k in range(1, nch):
            opsk = make_chunk_ops(k, bounds[k][0], bounds[k][1], ucs)
            w0 = c1st[k - 1]
            wn = bounds[k - 1][1] // BC
            for i, op in enumerate(opsk):
                emit_at[w0 + i * wn // len(opsk)].append(op)

        # ================= scan (GEMM chunks interleaved) =================
        cT = spool.tile([128, 2, BC], f32, tag="cT")
        hT_r = spool.tile([128, 2, BC], mdt, tag="hT_r")
        cT_r = spool.tile([128, 2, BC], mdt, tag="cT_r")
        nc.vector.memset(cT, 0.0)
        nc.vector.tensor_copy(out=hT_r, in_=cT)
        nc.vector.tensor_copy(out=cT_r, in_=cT)

        # initialize the psG bank's has_written bits with one whole-tile
        # start=True matmul; afterwards every step preloads u_t via DVE and
        # accumulates matmuls on top (start=False).
        psG_init = gsum2.tile([128, 8, BC], f32, tag="psG")
        psG_init2 = gsum2.tile([128, 8, BC], f32, tag="psG")
        nc.tensor.matmul(psG_init.rearrange("p a b -> p (a b)"),
                         wdb_row[:, 0:128], ones_r, start=True, stop=True)
        nc.tensor.matmul(psG_init2.rearrange("p a b -> p (a b)"),
                         wdb_row[:, 0:128], ones_r, start=True, stop=True)

        fbuf = None
        frch = None
        FSP = 32               # feat chunk grain (steps)
        for t in range(S):
            sc = t % FSP
            if sc == 0:
                fbuf = fpool.tile([128, FSP, 2, BC], f32, tag="fb")
                frch = frpool.tile([128, FSP, 2, BC], mdt, tag="fr")

            # c path: psA = W_d^T c + b_d (rank-1 bias matmul)
            psA = spsum.tile([128, 2, BC], f32, tag="psA")
            for m in range(2):
                for k in range(2):
                    nc.tensor.matmul(
                        psA[:, m], wds[:, k, m * 128:(m + 1) * 128],
                        cT_r[:, k], start=(k == 0), stop=False)
                nc.tensor.matmul(
                    psA[:, m], wdb_row[:, m * 128:(m + 1) * 128],
                    ones_r[:, 0:BC], start=False, stop=True)
            cs1 = spool.tile([128, 2, BC], f32, tag="cs1")
            nc.scalar.activation(out=cs1, in_=psA, func=AF.Tanh)
            cadj = spool.tile([128, 2, BC], f32, tag="cadj")
            nc.gpsimd.tensor_mul(
                cadj, cs1, tsb_all[:, t].unsqueeze(1).broadcast_to((128, 2, BC)))
            nc.gpsimd.tensor_add(cadj, cadj, cT)

            # h path
            psG = gsum2.tile([128, 8, BC], f32, tag="psG")
            uci = s2c[t]
            uof = t - c1st[uci]
            ut = ucs[uci][:, :, uof * BC:(uof + 1) * BC]
            gate = spool.tile([128, 8, BC], f32, tag="gate")
            # preload u_t into PSUM (off-chain), matmuls accumulate on top
            # (has_written set by the init matmul), sigmoid reads PSUM.
            nc.vector.tensor_copy(out=psG, in_=ut)
            for m in range(8):
                gm = GPERM[m]
                for k in range(2):
                    nc.tensor.matmul(
                        psG[:, m], was[:, k, gm * 128:(gm + 1) * 128],
                        hT_r[:, k], start=False, stop=(k == 1),
                        skip_group_check=True)
            # slots 0:6 = f,i,g (feed c_new); slots 6:8 = o
            nc.scalar.activation(out=gate[:, 0:6], in_=psG[:, 0:6],
                                 func=AF.Sigmoid)
            nc.scalar.activation(out=gate[:, 6:8], in_=psG[:, 6:8],
                                 func=AF.Sigmoid)

            # c_new = f*c_adj + i*g ; h = o*tanh(c_new)
            t1 = spool.tile([128, 2, BC], f32, tag="t1")
            nc.vector.tensor_mul(t1, gate[:, 0:2], cadj)
            t2 = spool.tile([128, 2, BC], f32, tag="t2")
            nc.vector.tensor_mul(t2, gate[:, 2:4], gate[:, 4:6])
            cT = spool.tile([128, 2, BC], f32, tag="cT")
            nc.vector.tensor_add(cT, t1, t2)
            tnh = spool.tile([128, 2, BC], f32, tag="tnh")
            nc.scalar.activation(out=tnh, in_=cT, func=AF.Tanh)
            hT_r = frch[:, sc]
            nc.vector.tensor_mul(hT_r, gate[:, 6:8], tnh)
            nc.gpsimd.tensor_mul(fbuf[:, sc], gate[:, 6:8], tnh)
            cT_r = spool.tile([128, 2, BC], mdt, tag="cT_r")
            nc.vector.tensor_copy(out=cT_r, in_=cT)

            # emit this step's share of the interleaved GEMM stream
            for op in emit_at[t]:
                op()

            if sc == FSP - 1:
                c = t // FSP
                FW = FSP * BC
                nc.sync.dma_start(
                    out=featT[:, c * FW * 2:(c + 1) * FW * 2], in_=fbuf)
                psO = spsum.tile([1, FW], f32, tag="psO")
                for j in range(2):
                    nc.tensor.matmul(psO, clws[:, j], frch[:, :, j],
                                     start=(j == 0), stop=(j == 1))
                oseg = frpool.tile([1, FW], f32, tag="oseg")
                nc.scalar.activation(out=oseg, in_=psO, func=AF.Identity,
                                     bias=clbs[0:1, 0:1], scale=1.0)
                nc.sync.dma_start(out=outT[:, c * FW:(c + 1) * FW], in_=oseg)


def kernel(x, stages, timestamps, se_w1, se_b1, se_w2, se_b2, se_w3, se_b3,
           W_all_w, W_all_b, U_all_w, U_all_b, W_d_w, W_d_b, cls_w, cls_b,
           _cfg=None, _nc=None):
    cfg = dict(DEFAULT_CFG, **(_cfg or {}))
    np_mdt = _np_mmdt(cfg)

    x = np.asarray(x, dtype=np.float32)
    stages = np.asarray(stages, dtype=np.float32)
    timestamps = np.asarray(timestamps, dtype=np.float32)

    nc = _nc if _nc is not None else _build(_cfg)

    def cvt(a):
        return np.ascontiguousarray(
            np.asarray(a, dtype=np.float32).astype(np_mdt))

    shared = {
        "w1": cvt(se_w1), "b1": np.ascontiguousarray(se_b1, dtype=np.float32),
        "w2": cvt(se_w2), "b2": np.ascontiguousarray(se_b2, dtype=np.float32),
        "w3": cvt(se_w3), "b3": np.ascontiguousarray(se_b3, dtype=np.float32),
        "uw": cvt(U_all_w),
        "ub": np.ascontiguousarray(
            np.asarray(U_all_b, dtype=np.float32)
            + np.asarray(W_all_b, dtype=np.float32)),
        "waw": cvt(W_all_w), "wdw": cvt(W_d_w), "wdb": cvt(W_d_b),
        "clw": cvt(cls_w),
        "clb": np.ascontiguousarray(cls_b, dtype=np.float32),
    }
    in_maps = []
    for k in range(NCORES):
        xs = x[k * BC:(k + 1) * BC]
        ss = stages[k * BC:(k + 1) * BC]
        ts = timestamps[k * BC:(k + 1) * BC]
        m = dict(shared)
        m["xT"] = cvt(xs.transpose(2, 1, 0).reshape(DX, R))
        m["stT"] = cvt(ss.transpose(2, 1, 0).reshape(DST, R))
        m["ts1"] = np.ascontiguousarray(ts.T) - 1.0
        in_maps.append(m)

    res = bass_utils.run_bass_kernel_spmd(nc, in_maps, list(range(NCORES)))

    feat = np.empty((B, S, H), dtype=np.float32)
    out = np.empty((B, S, 1), dtype=np.float32)
    for k in range(NCORES):
        ft = res.results[k]["featT"].reshape(128, S, 2, BC)
        feat[k * BC:(k + 1) * BC] = ft.transpose(3, 1, 2, 0).reshape(BC, S, H)
        ot = res.results[k]["outT"].reshape(S, BC)
        out[k * BC:(k + 1) * BC] = ot.T.reshape(BC, S, 1)
    return out.reshape(B * S, 1), feat.reshape(B * S, H)
